# revision 32
# baseline (speedup 1.0000x reference)
"""CentroidSeparationLoss on 8 Trainium2 NeuronCores.

Strategy (data-parallel over the batch):
  - Shard the 1M rows across 8 cores (125056 rows/core, tail zero-padded with
    out-of-range targets so padded rows contribute nothing).
  - Features stream HBM->SBUF as f32 via HWDGE DMAs alternating between
    the two HW-DGE rings (sync + scalar): one ring serializes its DMAs with
    a ~1.9us per-DMA gap (6.5us per 2MB tile); two rings overlap the gaps.
    HWDGE descriptors are RTL-generated (no SBUF descriptor rings), so the
    stream is immune to the SWDGE slow-engine backlog that randomly gates
    tile completions and trickles the stream tail (SWDGE cast-DMAs hit it
    on 1-3 random cores per run).
  - DVE casts each landed tile f32->bf16 (2x copy mode, ~2.2us/tile); ACT
    squares the bf16 copy (Square+accum, ~3.4us/tile). With the i32 one-hot
    (~2.3us) both engines stay ~15% under the 5.3us stream tile period.
  - Targets arrive in ONE blocked [128, 977] i32 DMA (host pre-blocks the
    layout to match the per-tile (p, j) row mapping); the per-tile one-hot
    is a DVE is_equal against an on-device iota (i32 in, bf16 out).
  - Per tile, one streaming pass computes:
      * per-class sums^T [128,64] via PE:  psum += f_tile.T @ onehot_tile
        (f stationary bf16 -> fast weight load; onehot moving, N=64)
      * total sum-of-squares: ACT Square+accum into columns of the merged
        output tile.
  - Tiles shrink toward the end ([32]*29 + [24, 16, 8, 1]) so the
    post-stream drain is one small tile's compute + one output DMA.
  - Host computes per-class counts directly from the int32 targets
    (np.bincount) and does the final reduction: centers, the closed-form
    intra loss (SSQ - sum_c ||sums_c||^2 / n_c)/B, and the pairwise inter
    hinge on the 64 centers -- identical to the reference up to bf16
    rounding of the streamed features.
"""

import numpy as np

import concourse.bacc as bacc
import concourse.mybir as mybir
import concourse.tile as tile
from concourse.bass_utils import run_bass_kernel_spmd

P = 128          # partitions
C = 64           # classes
D = 128          # feature dim
NJ = 32          # subtiles (of 128 rows) per big tile
N_CORES = 8
B_FULL = 1_000_000
ROWS_PER_CORE = 125056           # 977 subtiles of 128 rows
TILES_NJ = [NJ] * 29 + [24, 16, 8, 1]   # sum = 977
N_TILES = len(TILES_NJ)
S_TOTAL = sum(TILES_NJ)
MARGIN = 2.0
PAD_CLASS = C                    # out-of-range target for padded rows
SQ_ACT = 32                      # subtiles squared on ACT (keep DVE nearly
                                 # idle: its 2-port perf modes lock the SBUF
                                 # ports SWDGE descriptor fetches need, which
                                 # starves SDMA engines 7/15 and trickles the
                                 # stream tail)
PREFETCH = 6                     # f32 feature tiles in flight (2MB each)
N_DRAIN = 4                      # last tiles: split cast/squares ACT|DVE

F32 = mybir.dt.float32
BF16 = mybir.dt.bfloat16
I32 = mybir.dt.int32


def kernel_body(tc, outs, ins, tiles_nj):
    nc = tc.nc
    feat, tgt = ins
    (out_main,) = outs
    n_tiles = len(tiles_nj)
    nj_max = max(tiles_nj)
    n_mm = sum(tiles_nj)
    from collections import Counter
    nj_count = Counter(tiles_nj)

    row0s, s0s = [], []
    r = s = 0
    for nj in tiles_nj:
        row0s.append(r)
        s0s.append(s)
        r += P * nj
        s += nj

    with (
        tc.tile_pool(name="pf32", bufs=PREFETCH) as pf32,
        tc.tile_pool(name="pfaug", bufs=3) as pfaug,
        tc.tile_pool(name="poh", bufs=4) as poh,
        tc.tile_pool(name="psq", bufs=2) as psq,
        tc.tile_pool(name="pconst", bufs=1) as pconst,
        tc.tile_pool(name="pout", bufs=1) as pout,
        tc.tile_pool(name="ppsum", bufs=1, space="PSUM") as ppsum,
    ):
        # targets: one blocked DMA on the scalar HWDGE ring so the sync
        # ring carries only the feature stream
        tgt_i = pconst.tile([P, S_TOTAL], I32)
        nc.scalar.dma_start(tgt_i[:, :], tgt[:, :])

        def f32_tile(t):
            nj = tiles_nj[t]
            sfx = "" if nj == NJ else f"_tail{nj}"
            tb = None if nj == NJ else nj_count[nj]
            return pf32.tile([P, nj, D], F32, name="f32t" + sfx,
                             tag="f32t" + sfx, bufs=tb)

        # prefetch: issue the first PREFETCH feature DMAs up front
        f32s = {}
        for t in range(min(PREFETCH, n_tiles)):
            nj = tiles_nj[t]
            fap = feat[row0s[t] : row0s[t] + P * nj, :].rearrange(
                "(p j) d -> p j d", p=P, j=nj
            )
            f32s[t] = f32_tile(t)
            eng = nc.sync if t % 2 == 0 else nc.scalar
            eng.dma_start(f32s[t][:, :, :], fap)

        # iota over classes (i32: the one-hot compare stays in DVE 1x mode —
        # 2-port DVE perf modes starve SWDGE descriptor fetch); emitted after
        # the prefetch issues so it doesn't delay the stream start
        iota_i = pconst.tile([P, 1, C], I32)
        nc.gpsimd.iota(
            iota_i[:, :, :],
            pattern=[[0, 1], [1, C]],
            base=0,
            channel_multiplier=0,
        )

        # merged output: [:, 0:64] sums^T (copied from psum at the end),
        # [:, 64 + 2t] ACT ssq col, [:, 64 + 2t + 1] DVE ssq col per tile
        W = C + 2 * n_tiles
        main_sb = pout.tile([P, W], F32)
        nc.gpsimd.memset(main_sb[:, :], 0.0)

        psum_sumsT = ppsum.tile([P, C], F32)

        mm_idx = 0
        for t in range(n_tiles):
            nj = tiles_nj[t]
            s0 = s0s[t]
            f32t = f32s.pop(t)
            drain = t >= n_tiles - N_DRAIN

            sfx = "" if nj == NJ else f"_tail{nj}"
            tb = None if nj == NJ else nj_count[nj]
            if drain:
                sfx = f"_dr{t}"
                tb = 1

            # f32 -> bf16 cast: DVE 2x copy (cheaper per element than ACT);
            # drain tiles split halves across both engines, crossed with the
            # squares so the post-stream chain runs in parallel
            faug = pfaug.tile([P, nj, D], BF16, name="faug" + sfx,
                              tag="faug" + sfx, bufs=tb)
            c_sp = (nj + 1) // 2 if drain else 0
            if c_sp:
                nc.scalar.activation(
                    faug[:, 0:c_sp, :],
                    f32t[:, 0:c_sp, :],
                    mybir.ActivationFunctionType.Copy,
                )
            if nj > c_sp:
                nc.vector.tensor_copy(faug[:, c_sp:nj, :],
                                      f32t[:, c_sp:nj, :])

            oh = poh.tile([P, nj, C], BF16, tag="oh" + sfx, bufs=tb)
            nc.vector.tensor_tensor(
                oh[:, :, :],
                iota_i[:, 0:1, :].broadcast_to([P, nj, C]),
                tgt_i[:, s0 : s0 + nj].broadcast_to([P, nj, C]),
                op=mybir.AluOpType.is_equal,
            )

            # sum of squares: all on ACT (reads the bf16 copy); drain
            # tiles put the DVE-cast half on ACT and the ACT-cast half on
            # DVE stt so the two chains cross and finish together
            s_sp = nj if not drain else nj - (nj + 1) // 2
            if s_sp:
                sqa = psq.tile([P, s_sp, D], BF16, tag="sqa" + sfx,
                               bufs=1 if not drain else tb)
                nc.scalar.activation(
                    sqa[:, :, :],
                    faug[:, nj - s_sp : nj, :],
                    mybir.ActivationFunctionType.Square,
                    accum_out=main_sb[:, C + 2 * t : C + 2 * t + 1],
                )
            if nj > s_sp:
                sqb = psq.tile([P, nj - s_sp, D], BF16, tag="sqb" + sfx,
                               bufs=tb)
                nc.vector.scalar_tensor_tensor(
                    out=sqb[:, :, :],
                    in0=faug[:, 0 : nj - s_sp, :],
                    scalar=1.0,
                    in1=faug[:, 0 : nj - s_sp, :],
                    op0=mybir.AluOpType.mult,
                    op1=mybir.AluOpType.mult,
                    accum_out=main_sb[:, C + 2 * t + 1 : C + 2 * t + 2],
                )

            for j in range(nj):
                nc.tensor.matmul(
                    psum_sumsT[:, :],
                    lhsT=faug[:, j, :],
                    rhs=oh[:, j, :],
                    start=(mm_idx == 0),
                    stop=(mm_idx == n_mm - 1),
                )
                mm_idx += 1

            tn = t + PREFETCH
            if tn < n_tiles:
                njn = tiles_nj[tn]
                fap = feat[row0s[tn] : row0s[tn] + P * njn, :].rearrange(
                    "(p j) d -> p j d", p=P, j=njn
                )
                f32s[tn] = f32_tile(tn)
                eng = nc.sync if tn % 2 == 0 else nc.scalar
                eng.dma_start(f32s[tn][:, :, :], fap)

        nc.vector.tensor_copy(main_sb[:, 0:C], psum_sumsT[:, :])
        nc.sync.dma_start(out_main[:, :], main_sb[:, :])


def build_program(tiles_nj):
    # Bacc (not raw Bass): its compile() runs generate_event_semaphores,
    # which splits multi-semaphore waits into EventSemaphore instructions —
    # TRN2 instructions (notably direct-2D DMAs) carry at most one wait.
    nc = bacc.Bacc()
    n_tiles = len(tiles_nj)
    rows = P * sum(tiles_nj)
    feat = nc.dram_tensor("features", [rows, D], F32, kind="ExternalInput")
    tgt = nc.dram_tensor("targets", [P, S_TOTAL], I32, kind="ExternalInput")
    out_main = nc.dram_tensor("out_main", [P, C + 2 * n_tiles], F32,
                              kind="ExternalOutput")
    with tile.TileContext(nc) as tc:
        kernel_body(
            tc,
            (out_main[:, :],),
            (feat[:, :], tgt[:, :]),
            tiles_nj,
        )
    nc.compile()
    return nc


_PROGRAM = None


def _get_program():
    global _PROGRAM
    if _PROGRAM is None:
        _PROGRAM = build_program(TILES_NJ)
    return _PROGRAM


def make_in_maps(features, targets):
    features = np.asarray(features)
    targets = np.asarray(targets)
    if features.dtype != np.float32:
        features = features.astype(np.float32)
    if targets.dtype != np.int32:
        targets = targets.astype(np.int32)
    in_maps = []
    b = features.shape[0]
    for i in range(N_CORES):
        lo = i * ROWS_PER_CORE
        hi = min((i + 1) * ROWS_PER_CORE, b)
        f = features[lo:hi]
        t = targets[lo:hi]
        pad = ROWS_PER_CORE - (hi - lo)
        if pad:
            f = np.concatenate([f, np.zeros((pad, D), np.float32)])
            t = np.concatenate([t, np.full((pad,), PAD_CLASS, np.int32)])
        # block the targets to the per-tile (p, j) layout: tile t's subtile
        # column s0+j on partition p holds target[row0 + p*nj + j]
        tb = np.empty((P, S_TOTAL), np.int32)
        r = s = 0
        for nj in TILES_NJ:
            tb[:, s : s + nj] = t[r : r + P * nj].reshape(P, nj)
            r += P * nj
            s += nj
        in_maps.append({"features": f, "targets": tb})
    return in_maps


def reduce_partials(main_parts, counts, b):
    """main_parts: [P, C + 2*n_tiles] per core (sums^T cols 0:64, then ssq
    accumulator columns); counts: exact per-class counts from the host."""
    sums = np.zeros((C, D), np.float64)
    ssq = 0.0
    for m in main_parts:
        m = m.astype(np.float64)
        sums += m[:, 0:C].T
        ssq += float(m[:, C:].sum())

    counts_c = np.maximum(counts, 1.0)
    centers = sums / counts_c[:, None]
    intra = (
        ssq
        - 2.0 * float((sums * centers).sum())
        + float((counts * (centers**2).sum(axis=1)).sum())
    ) / b

    gram = centers @ centers.T
    n2 = np.diag(gram)
    d2 = n2[:, None] + n2[None, :] - 2.0 * gram
    hinge = np.maximum(MARGIN - d2, 0.0)
    w = np.ones((C, C))
    w[1, 2] = 2.0
    upper = np.triu(np.ones((C, C)), k=1)
    n_pairs = C * (C - 1) // 2
    inter = float((w * hinge * upper).sum()) / n_pairs
    return np.float32(intra + inter)


def exec_prestaged(nc, in_maps):
    """Execute the prebuilt Bass module on 8 cores with explicit input
    staging: device_put every shard and block until it lands in HBM BEFORE
    launching. The stock jit path lets a device start executing while other
    devices' host->HBM input transfers are still in flight on shared HBM
    stacks, which randomly robs 15-20% of stream bandwidth from 1-3 cores
    per run. Same lowering as concourse.bass2jax.run_bass_via_pjrt.
    """
    import jax
    from jax.experimental.shard_map import shard_map
    from jax.sharding import Mesh, NamedSharding, PartitionSpec

    from concourse import bass2jax as b2j

    n_cores = len(in_maps)
    b2j.install_neuronx_cc_hook()

    partition_name = (
        nc.partition_id_tensor.name if nc.partition_id_tensor else None
    )
    in_names, out_names, out_avals, zero_outs = [], [], [], []
    for alloc in nc.m.functions[0].allocations:
        if not isinstance(alloc, mybir.MemoryLocationSet):
            continue
        name = alloc.memorylocations[0].name
        if alloc.kind == "ExternalInput":
            if name != partition_name:
                in_names.append(name)
        elif alloc.kind == "ExternalOutput":
            out_names.append(name)
            shape = tuple(alloc.tensor_shape)
            dtype = mybir.dt.np(alloc.dtype)
            out_avals.append(jax.core.ShapedArray(shape, dtype))
            zero_outs.append(np.zeros(shape, dtype))
    n_params = len(in_names)
    n_outs = len(out_avals)
    in_names.extend(out_names)
    if partition_name is not None:
        in_names.append(partition_name)
    donate = tuple(range(n_params, n_params + n_outs))

    def _body(*args):
        operands = list(args)
        if partition_name is not None:
            operands.append(b2j.partition_id_tensor())
        outs = b2j._bass_exec_p.bind(
            *operands,
            out_avals=tuple(out_avals),
            in_names=tuple(in_names),
            out_names=tuple(out_names),
            lowering_input_output_aliases=(),
            sim_require_finite=True,
            sim_require_nnan=True,
            nc=nc,
        )
        return tuple(outs)

    devices = jax.devices()[:n_cores]
    mesh = Mesh(np.asarray(devices), ("core",))
    in_specs = (PartitionSpec("core"),) * (n_params + n_outs)
    out_specs = (PartitionSpec("core"),) * len(out_names)
    sharded = jax.jit(
        shard_map(_body, mesh=mesh, in_specs=in_specs, out_specs=out_specs,
                  check_rep=False),
        donate_argnums=donate,
        keep_unused=True,
    )
    per_core = [
        [np.asarray(m[name]) for name in in_names[:n_params]] for m in in_maps
    ]
    concat_in = [
        np.concatenate([per_core[c][i] for c in range(n_cores)], axis=0)
        for i in range(n_params)
    ]
    concat_zeros = [
        np.zeros((n_cores * z.shape[0], *z.shape[1:]), z.dtype)
        for z in zero_outs
    ]
    sharding = NamedSharding(mesh, PartitionSpec("core"))
    staged = jax.device_put(concat_in + concat_zeros, [sharding] * (n_params + n_outs))
    jax.block_until_ready(staged)
    out_arrs = sharded(*staged)
    jax.block_until_ready(out_arrs)
    return [
        {
            name: np.asarray(out_arrs[i]).reshape(n_cores, *out_avals[i].shape)[c]
            for i, name in enumerate(out_names)
        }
        for c in range(n_cores)
    ]


def run(features, targets, trace=False, trace_cores=None, prestage=True, **kw):
    nc = _get_program()
    in_maps = make_in_maps(features, targets)
    if prestage and not trace:
        try:
            results = exec_prestaged(nc, in_maps)
        except Exception:
            res = run_bass_kernel_spmd(
                nc, in_maps, core_ids=list(range(N_CORES))
            )
            results = res.results
            targets = np.asarray(targets)
            counts = np.bincount(
                targets.astype(np.int64).reshape(-1), minlength=C
            )[:C].astype(np.float64)
            out = reduce_partials(
                [r["out_main"] for r in results],
                counts,
                np.asarray(features).shape[0],
            )
            return out, res

        class _Res:
            pass

        res = _Res()
        res.results = results
        res.exec_time_ns = None
        res.mean_exec_time_ns = None
        res.max_exec_time_core_id = None
        res.instructions_and_trace = None
    else:
        res = run_bass_kernel_spmd(
            nc,
            in_maps,
            core_ids=list(range(N_CORES)),
            trace=trace,
            trace_cores=trace_cores,
            **kw,
        )
    targets = np.asarray(targets)
    counts = np.bincount(
        targets.astype(np.int64).reshape(-1), minlength=C
    )[:C].astype(np.float64)
    out = reduce_partials(
        [r["out_main"] for r in res.results],
        counts,
        np.asarray(features).shape[0],
    )
    return out, res


def kernel(features, targets):
    out, _ = run(features, targets)
    return np.array(out, dtype=np.float32)


# revision 36
# speedup vs baseline: 1.0840x; 1.0840x over previous
"""CentroidSeparationLoss on 8 Trainium2 NeuronCores.

Strategy (data-parallel over the batch):
  - Shard the 1M rows across 8 cores (125056 rows/core, tail zero-padded with
    out-of-range targets so padded rows contribute nothing).
  - Features stream HBM->SBUF via SWDGE cast-DMAs (f32 read, bf16 write): the
    dtype conversion happens inline in the SDMA engines, so no compute engine
    spends time casting and SBUF tiles are half the size (deep prefetch that
    rides out HBM arbitration jitter).
  - Targets arrive in ONE blocked [128, 977] i32 DMA (host pre-blocks the
    layout to match the per-tile (p, j) row mapping); the per-tile one-hot
    is a DVE is_equal against an on-device iota (i32 in, bf16 out).
  - Per tile, one streaming pass computes:
      * per-class sums^T [128,64] via PE:  psum += f_tile.T @ onehot_tile
        (f stationary bf16 -> fast weight load; onehot moving, N=64)
      * total sum-of-squares: ACT Square+accum (all 32 subtiles; DVE is
        kept nearly idle so its port activity cannot starve SWDGE
        descriptor fetches) into columns of the merged output tile.
  - Tiles shrink toward the end ([32]*29 + [24, 16, 8, 1]) so the
    post-stream drain is one small tile's compute + one output DMA.
  - Host computes per-class counts directly from the int32 targets
    (np.bincount) and does the final reduction: centers, the closed-form
    intra loss (SSQ - sum_c ||sums_c||^2 / n_c)/B, and the pairwise inter
    hinge on the 64 centers -- identical to the reference up to bf16
    rounding of the streamed features.
"""

import numpy as np

import concourse.bacc as bacc
import concourse.mybir as mybir
import concourse.tile as tile
from concourse.bass_utils import run_bass_kernel_spmd

P = 128          # partitions
C = 64           # classes
D = 128          # feature dim
NJ = 32          # subtiles (of 128 rows) per big tile
N_CORES = 8
B_FULL = 1_000_000
ROWS_PER_CORE = 125056           # 977 subtiles of 128 rows
TILES_NJ = [NJ] * 29 + [24, 16, 8, 1]   # sum = 977
N_TILES = len(TILES_NJ)
S_TOTAL = sum(TILES_NJ)
MARGIN = 2.0
PAD_CLASS = C                    # out-of-range target for padded rows
SQ_ACT = 32                      # subtiles squared on ACT (keep DVE nearly
                                 # idle: its 2-port perf modes lock the SBUF
                                 # ports SWDGE descriptor fetches need, which
                                 # starves SDMA engines 7/15 and trickles the
                                 # stream tail)
PREFETCH = 14                    # feature tiles in flight (bf16, 1MB each)

F32 = mybir.dt.float32
BF16 = mybir.dt.bfloat16
I32 = mybir.dt.int32


def kernel_body(tc, outs, ins, tiles_nj):
    nc = tc.nc
    feat, tgt = ins
    (out_main,) = outs
    n_tiles = len(tiles_nj)
    nj_max = max(tiles_nj)
    n_mm = sum(tiles_nj)
    from collections import Counter
    nj_count = Counter(tiles_nj)

    row0s, s0s = [], []
    r = s = 0
    for nj in tiles_nj:
        row0s.append(r)
        s0s.append(s)
        r += P * nj
        s += nj

    with (
        tc.tile_pool(name="pfaug", bufs=PREFETCH) as pfaug,
        tc.tile_pool(name="poh", bufs=6) as poh,
        tc.tile_pool(name="psq", bufs=2) as psq,
        tc.tile_pool(name="pconst", bufs=1) as pconst,
        tc.tile_pool(name="pout", bufs=1) as pout,
        tc.tile_pool(name="ppsum", bufs=1, space="PSUM") as ppsum,
    ):
        # targets: one blocked DMA on the HWDGE ring (tiny, lands early)
        tgt_i = pconst.tile([P, S_TOTAL], I32)
        nc.sync.dma_start(tgt_i[:, :], tgt[:, :])

        def faug_tile(t):
            nj = tiles_nj[t]
            sfx = "" if nj == NJ else f"_tail{nj}"
            tb = None if nj == NJ else nj_count[nj]
            return pfaug.tile([P, nj, D], BF16, name="faug" + sfx,
                              tag="faug" + sfx, bufs=tb)

        # prefetch: issue the first PREFETCH feature cast-DMAs up front
        faugs = {}
        for t in range(min(PREFETCH, n_tiles)):
            nj = tiles_nj[t]
            fap = feat[row0s[t] : row0s[t] + P * nj, :].rearrange(
                "(p j) d -> p j d", p=P, j=nj
            )
            faugs[t] = faug_tile(t)
            nc.gpsimd.dma_start(faugs[t][:, :, :], fap)

        # iota over classes (i32: the one-hot compare stays in DVE 1x mode —
        # 2-port DVE perf modes starve SWDGE descriptor fetch); emitted after
        # the prefetch issues so it doesn't delay the stream start
        iota_i = pconst.tile([P, nj_max, C], I32)
        nc.gpsimd.iota(
            iota_i[:, :, :],
            pattern=[[0, nj_max], [1, C]],
            base=0,
            channel_multiplier=0,
        )

        # merged output: [:, 0:64] sums^T (copied from psum at the end),
        # [:, 64 + 2t] ACT ssq col, [:, 64 + 2t + 1] DVE ssq col per tile
        W = C + 2 * n_tiles
        main_sb = pout.tile([P, W], F32)
        nc.gpsimd.memset(main_sb[:, :], 0.0)

        psum_sumsT = ppsum.tile([P, C], F32)

        mm_idx = 0
        for t in range(n_tiles):
            nj = tiles_nj[t]
            s0 = s0s[t]
            faug = faugs.pop(t)

            sfx = "" if nj == NJ else f"_tail{nj}"
            tb = None if nj == NJ else nj_count[nj]
            oh = poh.tile([P, nj, C], BF16, tag="oh" + sfx, bufs=tb)
            nc.vector.tensor_tensor(
                oh[:, :, :],
                iota_i[:, 0:nj, :],
                tgt_i[:, s0 : s0 + nj].broadcast_to([P, nj, C]),
                op=mybir.AluOpType.is_equal,
            )

            # sum of squares: ACT leading subtiles, DVE the rest. For the
            # last few tiles split half/half so the post-stream squares run
            # on both engines in parallel (shortest drain).
            if t >= n_tiles - 5:
                s_sp = nj // 2
                sfx = f"_dr{t}"
                tb = 1
            else:
                s_sp = min(nj, SQ_ACT)
            if s_sp:
                sqa = psq.tile([P, s_sp, D], BF16, tag="sqa" + sfx, bufs=tb)
                nc.scalar.activation(
                    sqa[:, :, :],
                    faug[:, 0:s_sp, :],
                    mybir.ActivationFunctionType.Square,
                    accum_out=main_sb[:, C + 2 * t : C + 2 * t + 1],
                )
            if nj > s_sp:
                sqb = psq.tile([P, nj - s_sp, D], BF16, tag="sqb" + sfx,
                               bufs=tb)
                nc.vector.scalar_tensor_tensor(
                    out=sqb[:, :, :],
                    in0=faug[:, s_sp:nj, :],
                    scalar=1.0,
                    in1=faug[:, s_sp:nj, :],
                    op0=mybir.AluOpType.mult,
                    op1=mybir.AluOpType.mult,
                    accum_out=main_sb[:, C + 2 * t + 1 : C + 2 * t + 2],
                )

            for j in range(nj):
                nc.tensor.matmul(
                    psum_sumsT[:, :],
                    lhsT=faug[:, j, :],
                    rhs=oh[:, j, :],
                    start=(mm_idx == 0),
                    stop=(mm_idx == n_mm - 1),
                )
                mm_idx += 1

            tn = t + PREFETCH
            if tn < n_tiles:
                njn = tiles_nj[tn]
                fap = feat[row0s[tn] : row0s[tn] + P * njn, :].rearrange(
                    "(p j) d -> p j d", p=P, j=njn
                )
                faugs[tn] = faug_tile(tn)
                nc.gpsimd.dma_start(faugs[tn][:, :, :], fap)

        nc.vector.tensor_copy(main_sb[:, 0:C], psum_sumsT[:, :])
        nc.sync.dma_start(out_main[:, :], main_sb[:, :])


def build_program(tiles_nj):
    # Bacc (not raw Bass): its compile() runs generate_event_semaphores,
    # which splits multi-semaphore waits into EventSemaphore instructions —
    # TRN2 instructions (notably direct-2D DMAs) carry at most one wait.
    nc = bacc.Bacc()
    n_tiles = len(tiles_nj)
    rows = P * sum(tiles_nj)
    feat = nc.dram_tensor("features", [rows, D], F32, kind="ExternalInput")
    tgt = nc.dram_tensor("targets", [P, S_TOTAL], I32, kind="ExternalInput")
    out_main = nc.dram_tensor("out_main", [P, C + 2 * n_tiles], F32,
                              kind="ExternalOutput")
    with tile.TileContext(nc) as tc:
        kernel_body(
            tc,
            (out_main[:, :],),
            (feat[:, :], tgt[:, :]),
            tiles_nj,
        )
    nc.compile()
    return nc


_PROGRAM = None


def _get_program():
    global _PROGRAM
    if _PROGRAM is None:
        _PROGRAM = build_program(TILES_NJ)
    return _PROGRAM


def make_in_maps(features, targets):
    features = np.asarray(features)
    targets = np.asarray(targets)
    if features.dtype != np.float32:
        features = features.astype(np.float32)
    if targets.dtype != np.int32:
        targets = targets.astype(np.int32)
    in_maps = []
    b = features.shape[0]
    for i in range(N_CORES):
        lo = i * ROWS_PER_CORE
        hi = min((i + 1) * ROWS_PER_CORE, b)
        f = features[lo:hi]
        t = targets[lo:hi]
        pad = ROWS_PER_CORE - (hi - lo)
        if pad:
            f = np.concatenate([f, np.zeros((pad, D), np.float32)])
            t = np.concatenate([t, np.full((pad,), PAD_CLASS, np.int32)])
        # block the targets to the per-tile (p, j) layout: tile t's subtile
        # column s0+j on partition p holds target[row0 + p*nj + j]
        tb = np.empty((P, S_TOTAL), np.int32)
        r = s = 0
        for nj in TILES_NJ:
            tb[:, s : s + nj] = t[r : r + P * nj].reshape(P, nj)
            r += P * nj
            s += nj
        in_maps.append({"features": f, "targets": tb})
    return in_maps


def reduce_partials(main_parts, counts, b):
    """main_parts: [P, C + 2*n_tiles] per core (sums^T cols 0:64, then ssq
    accumulator columns); counts: exact per-class counts from the host."""
    sums = np.zeros((C, D), np.float64)
    ssq = 0.0
    for m in main_parts:
        m = m.astype(np.float64)
        sums += m[:, 0:C].T
        ssq += float(m[:, C:].sum())

    counts_c = np.maximum(counts, 1.0)
    centers = sums / counts_c[:, None]
    intra = (
        ssq
        - 2.0 * float((sums * centers).sum())
        + float((counts * (centers**2).sum(axis=1)).sum())
    ) / b

    gram = centers @ centers.T
    n2 = np.diag(gram)
    d2 = n2[:, None] + n2[None, :] - 2.0 * gram
    hinge = np.maximum(MARGIN - d2, 0.0)
    w = np.ones((C, C))
    w[1, 2] = 2.0
    upper = np.triu(np.ones((C, C)), k=1)
    n_pairs = C * (C - 1) // 2
    inter = float((w * hinge * upper).sum()) / n_pairs
    return np.float32(intra + inter)


def exec_prestaged(nc, in_maps):
    """Execute the prebuilt Bass module on 8 cores with explicit input
    staging: device_put every shard and block until it lands in HBM BEFORE
    launching. The stock jit path lets a device start executing while other
    devices' host->HBM input transfers are still in flight on shared HBM
    stacks, which randomly robs 15-20% of stream bandwidth from 1-3 cores
    per run. Same lowering as concourse.bass2jax.run_bass_via_pjrt.
    """
    import jax
    from jax.experimental.shard_map import shard_map
    from jax.sharding import Mesh, NamedSharding, PartitionSpec

    from concourse import bass2jax as b2j

    n_cores = len(in_maps)
    b2j.install_neuronx_cc_hook()

    partition_name = (
        nc.partition_id_tensor.name if nc.partition_id_tensor else None
    )
    in_names, out_names, out_avals, zero_outs = [], [], [], []
    for alloc in nc.m.functions[0].allocations:
        if not isinstance(alloc, mybir.MemoryLocationSet):
            continue
        name = alloc.memorylocations[0].name
        if alloc.kind == "ExternalInput":
            if name != partition_name:
                in_names.append(name)
        elif alloc.kind == "ExternalOutput":
            out_names.append(name)
            shape = tuple(alloc.tensor_shape)
            dtype = mybir.dt.np(alloc.dtype)
            out_avals.append(jax.core.ShapedArray(shape, dtype))
            zero_outs.append(np.zeros(shape, dtype))
    n_params = len(in_names)
    n_outs = len(out_avals)
    in_names.extend(out_names)
    if partition_name is not None:
        in_names.append(partition_name)
    donate = tuple(range(n_params, n_params + n_outs))

    def _body(*args):
        operands = list(args)
        if partition_name is not None:
            operands.append(b2j.partition_id_tensor())
        outs = b2j._bass_exec_p.bind(
            *operands,
            out_avals=tuple(out_avals),
            in_names=tuple(in_names),
            out_names=tuple(out_names),
            lowering_input_output_aliases=(),
            sim_require_finite=True,
            sim_require_nnan=True,
            nc=nc,
        )
        return tuple(outs)

    devices = jax.devices()[:n_cores]
    mesh = Mesh(np.asarray(devices), ("core",))
    in_specs = (PartitionSpec("core"),) * (n_params + n_outs)
    out_specs = (PartitionSpec("core"),) * len(out_names)
    sharded = jax.jit(
        shard_map(_body, mesh=mesh, in_specs=in_specs, out_specs=out_specs,
                  check_rep=False),
        donate_argnums=donate,
        keep_unused=True,
    )
    per_core = [
        [np.asarray(m[name]) for name in in_names[:n_params]] for m in in_maps
    ]
    concat_in = [
        np.concatenate([per_core[c][i] for c in range(n_cores)], axis=0)
        for i in range(n_params)
    ]
    concat_zeros = [
        np.zeros((n_cores * z.shape[0], *z.shape[1:]), z.dtype)
        for z in zero_outs
    ]
    sharding = NamedSharding(mesh, PartitionSpec("core"))
    staged = jax.device_put(concat_in + concat_zeros, [sharding] * (n_params + n_outs))
    jax.block_until_ready(staged)
    out_arrs = sharded(*staged)
    jax.block_until_ready(out_arrs)
    return [
        {
            name: np.asarray(out_arrs[i]).reshape(n_cores, *out_avals[i].shape)[c]
            for i, name in enumerate(out_names)
        }
        for c in range(n_cores)
    ]


def run(features, targets, trace=False, trace_cores=None, prestage=True, **kw):
    nc = _get_program()
    in_maps = make_in_maps(features, targets)
    if prestage and not trace:
        try:
            results = exec_prestaged(nc, in_maps)
        except Exception:
            res = run_bass_kernel_spmd(
                nc, in_maps, core_ids=list(range(N_CORES))
            )
            results = res.results
            targets = np.asarray(targets)
            counts = np.bincount(
                targets.astype(np.int64).reshape(-1), minlength=C
            )[:C].astype(np.float64)
            out = reduce_partials(
                [r["out_main"] for r in results],
                counts,
                np.asarray(features).shape[0],
            )
            return out, res

        class _Res:
            pass

        res = _Res()
        res.results = results
        res.exec_time_ns = None
        res.mean_exec_time_ns = None
        res.max_exec_time_core_id = None
        res.instructions_and_trace = None
    else:
        res = run_bass_kernel_spmd(
            nc,
            in_maps,
            core_ids=list(range(N_CORES)),
            trace=trace,
            trace_cores=trace_cores,
            **kw,
        )
    targets = np.asarray(targets)
    counts = np.bincount(
        targets.astype(np.int64).reshape(-1), minlength=C
    )[:C].astype(np.float64)
    out = reduce_partials(
        [r["out_main"] for r in res.results],
        counts,
        np.asarray(features).shape[0],
    )
    return out, res


def kernel(features, targets):
    out, _ = run(features, targets)
    return np.array(out, dtype=np.float32)


# revision 37
# speedup vs baseline: 1.0946x; 1.0097x over previous
"""CentroidSeparationLoss on 8 Trainium2 NeuronCores.

Strategy (data-parallel over the batch):
  - Shard the 1M rows across 8 cores (125056 rows/core, tail zero-padded with
    out-of-range targets so padded rows contribute nothing).
  - Features stream HBM->SBUF via SWDGE cast-DMAs (f32 read, bf16 write): the
    dtype conversion happens inline in the SDMA engines, so no compute engine
    spends time casting and SBUF tiles are half the size (deep prefetch that
    rides out HBM arbitration jitter).
  - Targets arrive in ONE blocked [128, 977] i32 DMA (host pre-blocks the
    layout to match the per-tile (p, j) row mapping); the per-tile one-hot
    is a DVE is_equal against an on-device iota (i32 in, bf16 out).
  - Per tile, one streaming pass computes:
      * per-class sums^T [128,64] via PE:  psum += f_tile.T @ onehot_tile
        (f stationary bf16 -> fast weight load; onehot moving, N=64)
      * total sum-of-squares: ACT Square+accum (all 32 subtiles; DVE is
        kept nearly idle so its port activity cannot starve SWDGE
        descriptor fetches) into columns of the merged output tile.
  - Tiles shrink toward the end ([32]*29 + [24, 16, 8, 1]) so the
    post-stream drain is one small tile's compute + one output DMA.
  - Host computes per-class counts directly from the int32 targets
    (np.bincount) and does the final reduction: centers, the closed-form
    intra loss (SSQ - sum_c ||sums_c||^2 / n_c)/B, and the pairwise inter
    hinge on the 64 centers -- identical to the reference up to bf16
    rounding of the streamed features.
"""

import numpy as np

import concourse.bacc as bacc
import concourse.mybir as mybir
import concourse.tile as tile
from concourse.bass_utils import run_bass_kernel_spmd

P = 128          # partitions
C = 64           # classes
D = 128          # feature dim
NJ = 64          # subtiles (of 128 rows) per big tile
N_CORES = 8
B_FULL = 1_000_000
ROWS_PER_CORE = 125056           # 977 subtiles of 128 rows
TILES_NJ = [NJ] * 14 + [32, 24, 16, 4, 1]   # sum = 977
N_TILES = len(TILES_NJ)
S_TOTAL = sum(TILES_NJ)
MARGIN = 2.0
PAD_CLASS = C                    # out-of-range target for padded rows
SQ_ACT = 32                      # subtiles squared on ACT (keep DVE nearly
                                 # idle: its 2-port perf modes lock the SBUF
                                 # ports SWDGE descriptor fetches need, which
                                 # starves SDMA engines 7/15 and trickles the
                                 # stream tail)
PREFETCH = 4                     # feature tiles in flight (bf16, 2MB each)

F32 = mybir.dt.float32
BF16 = mybir.dt.bfloat16
I32 = mybir.dt.int32


def kernel_body(tc, outs, ins, tiles_nj):
    nc = tc.nc
    feat, tgt = ins
    (out_main,) = outs
    n_tiles = len(tiles_nj)
    nj_max = max(tiles_nj)
    n_mm = sum(tiles_nj)
    from collections import Counter
    nj_count = Counter(tiles_nj)

    row0s, s0s = [], []
    r = s = 0
    for nj in tiles_nj:
        row0s.append(r)
        s0s.append(s)
        r += P * nj
        s += nj

    with (
        tc.tile_pool(name="pfaug", bufs=PREFETCH) as pfaug,
        tc.tile_pool(name="poh", bufs=3) as poh,
        tc.tile_pool(name="psq", bufs=2) as psq,
        tc.tile_pool(name="pconst", bufs=1) as pconst,
        tc.tile_pool(name="pout", bufs=1) as pout,
        tc.tile_pool(name="ppsum", bufs=1, space="PSUM") as ppsum,
    ):
        # targets: one blocked DMA on the HWDGE ring (tiny, lands early)
        tgt_i = pconst.tile([P, S_TOTAL], I32)
        nc.sync.dma_start(tgt_i[:, :], tgt[:, :])

        def faug_tile(t):
            nj = tiles_nj[t]
            sfx = "" if nj == NJ else f"_tail{nj}"
            tb = None if nj == NJ else nj_count[nj]
            return pfaug.tile([P, nj, D], BF16, name="faug" + sfx,
                              tag="faug" + sfx, bufs=tb)

        # prefetch: issue the first PREFETCH feature cast-DMAs up front
        faugs = {}
        for t in range(min(PREFETCH, n_tiles)):
            nj = tiles_nj[t]
            fap = feat[row0s[t] : row0s[t] + P * nj, :].rearrange(
                "(p j) d -> p j d", p=P, j=nj
            )
            faugs[t] = faug_tile(t)
            nc.gpsimd.dma_start(faugs[t][:, :, :], fap)

        # iota over classes (i32: the one-hot compare stays in DVE 1x mode —
        # 2-port DVE perf modes starve SWDGE descriptor fetch); emitted after
        # the prefetch issues so it doesn't delay the stream start
        nj_iota = min(nj_max, 32)
        iota_i = pconst.tile([P, nj_iota, C], I32)
        nc.gpsimd.iota(
            iota_i[:, :, :],
            pattern=[[0, nj_iota], [1, C]],
            base=0,
            channel_multiplier=0,
        )

        # merged output: [:, 0:64] sums^T (copied from psum at the end),
        # [:, 64 + 2t] ACT ssq col, [:, 64 + 2t + 1] DVE ssq col per tile
        W = C + 2 * n_tiles
        main_sb = pout.tile([P, W], F32)
        nc.gpsimd.memset(main_sb[:, :], 0.0)

        psum_sumsT = ppsum.tile([P, C], F32)

        mm_idx = 0
        for t in range(n_tiles):
            nj = tiles_nj[t]
            s0 = s0s[t]
            faug = faugs.pop(t)

            sfx = "" if nj == NJ else f"_tail{nj}"
            tb = None if nj == NJ else nj_count[nj]
            oh = poh.tile([P, nj, C], BF16, tag="oh" + sfx, bufs=tb)
            for a in range(0, nj, nj_iota):
                bnd = min(nj, a + nj_iota)
                nc.vector.tensor_tensor(
                    oh[:, a:bnd, :],
                    iota_i[:, 0 : bnd - a, :],
                    tgt_i[:, s0 + a : s0 + bnd].broadcast_to(
                        [P, bnd - a, C]
                    ),
                    op=mybir.AluOpType.is_equal,
                )

            # sum of squares: ACT leading subtiles, DVE the rest. For the
            # last few tiles split half/half so the post-stream squares run
            # on both engines in parallel (shortest drain).
            if t >= n_tiles - 5:
                s_sp = nj // 2
                sfx = f"_dr{t}"
                tb = 1
            else:
                s_sp = nj
            if s_sp and not (t >= n_tiles - 5):
                half = nj // 2
                for hi in range(2):
                    sqa = psq.tile([P, half, D], BF16, tag="sqa", bufs=2)
                    nc.scalar.activation(
                        sqa[:, :, :],
                        faug[:, hi * half : (hi + 1) * half, :],
                        mybir.ActivationFunctionType.Square,
                        accum_out=main_sb[
                            :, C + 2 * t + hi : C + 2 * t + hi + 1
                        ],
                    )
            elif s_sp:
                sqa = psq.tile([P, s_sp, D], BF16, tag="sqa" + sfx, bufs=tb)
                nc.scalar.activation(
                    sqa[:, :, :],
                    faug[:, 0:s_sp, :],
                    mybir.ActivationFunctionType.Square,
                    accum_out=main_sb[:, C + 2 * t : C + 2 * t + 1],
                )
            if nj > s_sp:
                sqb = psq.tile([P, nj - s_sp, D], BF16, tag="sqb" + sfx,
                               bufs=tb)
                nc.vector.scalar_tensor_tensor(
                    out=sqb[:, :, :],
                    in0=faug[:, s_sp:nj, :],
                    scalar=1.0,
                    in1=faug[:, s_sp:nj, :],
                    op0=mybir.AluOpType.mult,
                    op1=mybir.AluOpType.mult,
                    accum_out=main_sb[:, C + 2 * t + 1 : C + 2 * t + 2],
                )

            for j in range(nj):
                nc.tensor.matmul(
                    psum_sumsT[:, :],
                    lhsT=faug[:, j, :],
                    rhs=oh[:, j, :],
                    start=(mm_idx == 0),
                    stop=(mm_idx == n_mm - 1),
                )
                mm_idx += 1

            tn = t + PREFETCH
            if tn < n_tiles:
                njn = tiles_nj[tn]
                fap = feat[row0s[tn] : row0s[tn] + P * njn, :].rearrange(
                    "(p j) d -> p j d", p=P, j=njn
                )
                faugs[tn] = faug_tile(tn)
                nc.gpsimd.dma_start(faugs[tn][:, :, :], fap)

        nc.vector.tensor_copy(main_sb[:, 0:C], psum_sumsT[:, :])
        nc.sync.dma_start(out_main[:, :], main_sb[:, :])


def build_program(tiles_nj):
    # Bacc (not raw Bass): its compile() runs generate_event_semaphores,
    # which splits multi-semaphore waits into EventSemaphore instructions —
    # TRN2 instructions (notably direct-2D DMAs) carry at most one wait.
    nc = bacc.Bacc()
    n_tiles = len(tiles_nj)
    rows = P * sum(tiles_nj)
    feat = nc.dram_tensor("features", [rows, D], F32, kind="ExternalInput")
    tgt = nc.dram_tensor("targets", [P, S_TOTAL], I32, kind="ExternalInput")
    out_main = nc.dram_tensor("out_main", [P, C + 2 * n_tiles], F32,
                              kind="ExternalOutput")
    with tile.TileContext(nc) as tc:
        kernel_body(
            tc,
            (out_main[:, :],),
            (feat[:, :], tgt[:, :]),
            tiles_nj,
        )
    nc.compile()
    return nc


_PROGRAM = None


def _get_program():
    global _PROGRAM
    if _PROGRAM is None:
        _PROGRAM = build_program(TILES_NJ)
    return _PROGRAM


def make_in_maps(features, targets):
    features = np.asarray(features)
    targets = np.asarray(targets)
    if features.dtype != np.float32:
        features = features.astype(np.float32)
    if targets.dtype != np.int32:
        targets = targets.astype(np.int32)
    in_maps = []
    b = features.shape[0]
    for i in range(N_CORES):
        lo = i * ROWS_PER_CORE
        hi = min((i + 1) * ROWS_PER_CORE, b)
        f = features[lo:hi]
        t = targets[lo:hi]
        pad = ROWS_PER_CORE - (hi - lo)
        if pad:
            f = np.concatenate([f, np.zeros((pad, D), np.float32)])
            t = np.concatenate([t, np.full((pad,), PAD_CLASS, np.int32)])
        # block the targets to the per-tile (p, j) layout: tile t's subtile
        # column s0+j on partition p holds target[row0 + p*nj + j]
        tb = np.empty((P, S_TOTAL), np.int32)
        r = s = 0
        for nj in TILES_NJ:
            tb[:, s : s + nj] = t[r : r + P * nj].reshape(P, nj)
            r += P * nj
            s += nj
        in_maps.append({"features": f, "targets": tb})
    return in_maps


def reduce_partials(main_parts, counts, b):
    """main_parts: [P, C + 2*n_tiles] per core (sums^T cols 0:64, then ssq
    accumulator columns); counts: exact per-class counts from the host."""
    sums = np.zeros((C, D), np.float64)
    ssq = 0.0
    for m in main_parts:
        m = m.astype(np.float64)
        sums += m[:, 0:C].T
        ssq += float(m[:, C:].sum())

    counts_c = np.maximum(counts, 1.0)
    centers = sums / counts_c[:, None]
    intra = (
        ssq
        - 2.0 * float((sums * centers).sum())
        + float((counts * (centers**2).sum(axis=1)).sum())
    ) / b

    gram = centers @ centers.T
    n2 = np.diag(gram)
    d2 = n2[:, None] + n2[None, :] - 2.0 * gram
    hinge = np.maximum(MARGIN - d2, 0.0)
    w = np.ones((C, C))
    w[1, 2] = 2.0
    upper = np.triu(np.ones((C, C)), k=1)
    n_pairs = C * (C - 1) // 2
    inter = float((w * hinge * upper).sum()) / n_pairs
    return np.float32(intra + inter)


def exec_prestaged(nc, in_maps):
    """Execute the prebuilt Bass module on 8 cores with explicit input
    staging: device_put every shard and block until it lands in HBM BEFORE
    launching. The stock jit path lets a device start executing while other
    devices' host->HBM input transfers are still in flight on shared HBM
    stacks, which randomly robs 15-20% of stream bandwidth from 1-3 cores
    per run. Same lowering as concourse.bass2jax.run_bass_via_pjrt.
    """
    import jax
    from jax.experimental.shard_map import shard_map
    from jax.sharding import Mesh, NamedSharding, PartitionSpec

    from concourse import bass2jax as b2j

    n_cores = len(in_maps)
    b2j.install_neuronx_cc_hook()

    partition_name = (
        nc.partition_id_tensor.name if nc.partition_id_tensor else None
    )
    in_names, out_names, out_avals, zero_outs = [], [], [], []
    for alloc in nc.m.functions[0].allocations:
        if not isinstance(alloc, mybir.MemoryLocationSet):
            continue
        name = alloc.memorylocations[0].name
        if alloc.kind == "ExternalInput":
            if name != partition_name:
                in_names.append(name)
        elif alloc.kind == "ExternalOutput":
            out_names.append(name)
            shape = tuple(alloc.tensor_shape)
            dtype = mybir.dt.np(alloc.dtype)
            out_avals.append(jax.core.ShapedArray(shape, dtype))
            zero_outs.append(np.zeros(shape, dtype))
    n_params = len(in_names)
    n_outs = len(out_avals)
    in_names.extend(out_names)
    if partition_name is not None:
        in_names.append(partition_name)
    donate = tuple(range(n_params, n_params + n_outs))

    def _body(*args):
        operands = list(args)
        if partition_name is not None:
            operands.append(b2j.partition_id_tensor())
        outs = b2j._bass_exec_p.bind(
            *operands,
            out_avals=tuple(out_avals),
            in_names=tuple(in_names),
            out_names=tuple(out_names),
            lowering_input_output_aliases=(),
            sim_require_finite=True,
            sim_require_nnan=True,
            nc=nc,
        )
        return tuple(outs)

    devices = jax.devices()[:n_cores]
    mesh = Mesh(np.asarray(devices), ("core",))
    in_specs = (PartitionSpec("core"),) * (n_params + n_outs)
    out_specs = (PartitionSpec("core"),) * len(out_names)
    sharded = jax.jit(
        shard_map(_body, mesh=mesh, in_specs=in_specs, out_specs=out_specs,
                  check_rep=False),
        donate_argnums=donate,
        keep_unused=True,
    )
    per_core = [
        [np.asarray(m[name]) for name in in_names[:n_params]] for m in in_maps
    ]
    concat_in = [
        np.concatenate([per_core[c][i] for c in range(n_cores)], axis=0)
        for i in range(n_params)
    ]
    concat_zeros = [
        np.zeros((n_cores * z.shape[0], *z.shape[1:]), z.dtype)
        for z in zero_outs
    ]
    sharding = NamedSharding(mesh, PartitionSpec("core"))
    staged = jax.device_put(concat_in + concat_zeros, [sharding] * (n_params + n_outs))
    jax.block_until_ready(staged)
    out_arrs = sharded(*staged)
    jax.block_until_ready(out_arrs)
    return [
        {
            name: np.asarray(out_arrs[i]).reshape(n_cores, *out_avals[i].shape)[c]
            for i, name in enumerate(out_names)
        }
        for c in range(n_cores)
    ]


def run(features, targets, trace=False, trace_cores=None, prestage=True, **kw):
    nc = _get_program()
    in_maps = make_in_maps(features, targets)
    if prestage and not trace:
        try:
            results = exec_prestaged(nc, in_maps)
        except Exception:
            res = run_bass_kernel_spmd(
                nc, in_maps, core_ids=list(range(N_CORES))
            )
            results = res.results
            targets = np.asarray(targets)
            counts = np.bincount(
                targets.astype(np.int64).reshape(-1), minlength=C
            )[:C].astype(np.float64)
            out = reduce_partials(
                [r["out_main"] for r in results],
                counts,
                np.asarray(features).shape[0],
            )
            return out, res

        class _Res:
            pass

        res = _Res()
        res.results = results
        res.exec_time_ns = None
        res.mean_exec_time_ns = None
        res.max_exec_time_core_id = None
        res.instructions_and_trace = None
    else:
        res = run_bass_kernel_spmd(
            nc,
            in_maps,
            core_ids=list(range(N_CORES)),
            trace=trace,
            trace_cores=trace_cores,
            **kw,
        )
    targets = np.asarray(targets)
    counts = np.bincount(
        targets.astype(np.int64).reshape(-1), minlength=C
    )[:C].astype(np.float64)
    out = reduce_partials(
        [r["out_main"] for r in res.results],
        counts,
        np.asarray(features).shape[0],
    )
    return out, res


def kernel(features, targets):
    out, _ = run(features, targets)
    return np.array(out, dtype=np.float32)


# revision 38
# speedup vs baseline: 1.0947x; 1.0001x over previous
"""CentroidSeparationLoss on 8 Trainium2 NeuronCores.

Strategy (data-parallel over the batch):
  - Shard the 1M rows across 8 cores (125056 rows/core, tail zero-padded with
    out-of-range targets so padded rows contribute nothing).
  - Features stream HBM->SBUF via SWDGE cast-DMAs (f32 read, bf16 write): the
    dtype conversion happens inline in the SDMA engines, so no compute engine
    spends time casting and SBUF tiles are half the size (deep prefetch that
    rides out HBM arbitration jitter).
  - Targets arrive in ONE blocked [128, 977] i32 DMA (host pre-blocks the
    layout to match the per-tile (p, j) row mapping); the per-tile one-hot
    is a DVE is_equal against an on-device iota (i32 in, bf16 out).
  - Per tile, one streaming pass computes:
      * per-class sums^T [128,64] via PE:  psum += f_tile.T @ onehot_tile
        (f stationary bf16 -> fast weight load; onehot moving, N=64)
      * total sum-of-squares: ACT Square+accum (all 32 subtiles; DVE is
        kept nearly idle so its port activity cannot starve SWDGE
        descriptor fetches) into columns of the merged output tile.
  - Tiles shrink toward the end ([32]*29 + [24, 16, 8, 1]) so the
    post-stream drain is one small tile's compute + one output DMA.
  - Host computes per-class counts directly from the int32 targets
    (np.bincount) and does the final reduction: centers, the closed-form
    intra loss (SSQ - sum_c ||sums_c||^2 / n_c)/B, and the pairwise inter
    hinge on the 64 centers -- identical to the reference up to bf16
    rounding of the streamed features.
"""

import numpy as np

import concourse.bacc as bacc
import concourse.mybir as mybir
import concourse.tile as tile
from concourse.bass_utils import run_bass_kernel_spmd

P = 128          # partitions
C = 64           # classes
D = 128          # feature dim
NJ = 32          # subtiles (of 128 rows) per big tile
N_CORES = 8
B_FULL = 1_000_000
ROWS_PER_CORE = 125056           # 977 subtiles of 128 rows
TILES_NJ = [NJ] * 29 + [24, 16, 8, 1]   # sum = 977
N_TILES = len(TILES_NJ)
S_TOTAL = sum(TILES_NJ)
MARGIN = 2.0
PAD_CLASS = C                    # out-of-range target for padded rows
SQ_ACT = 32                      # subtiles squared on ACT (keep DVE nearly
                                 # idle: its 2-port perf modes lock the SBUF
                                 # ports SWDGE descriptor fetches need, which
                                 # starves SDMA engines 7/15 and trickles the
                                 # stream tail)
PREFETCH = 14                    # feature tiles in flight (bf16, 1MB each)

F32 = mybir.dt.float32
BF16 = mybir.dt.bfloat16
I32 = mybir.dt.int32


def kernel_body(tc, outs, ins, tiles_nj):
    nc = tc.nc
    feat, tgt = ins
    (out_main,) = outs
    n_tiles = len(tiles_nj)
    nj_max = max(tiles_nj)
    n_mm = sum(tiles_nj)
    from collections import Counter
    nj_count = Counter(tiles_nj)

    row0s, s0s = [], []
    r = s = 0
    for nj in tiles_nj:
        row0s.append(r)
        s0s.append(s)
        r += P * nj
        s += nj

    with (
        tc.tile_pool(name="pfaug", bufs=PREFETCH) as pfaug,
        tc.tile_pool(name="poh", bufs=6) as poh,
        tc.tile_pool(name="psq", bufs=2) as psq,
        tc.tile_pool(name="pconst", bufs=1) as pconst,
        tc.tile_pool(name="pout", bufs=1) as pout,
        tc.tile_pool(name="ppsum", bufs=1, space="PSUM") as ppsum,
    ):
        # targets: one blocked DMA on the HWDGE ring (tiny, lands early)
        tgt_i = pconst.tile([P, S_TOTAL], I32)
        nc.sync.dma_start(tgt_i[:, :], tgt[:, :])

        def faug_tile(t):
            nj = tiles_nj[t]
            sfx = "" if nj == NJ else f"_tail{nj}"
            tb = None if nj == NJ else nj_count[nj]
            return pfaug.tile([P, nj, D], BF16, name="faug" + sfx,
                              tag="faug" + sfx, bufs=tb)

        # prefetch: issue the first PREFETCH feature cast-DMAs up front
        faugs = {}
        for t in range(min(PREFETCH, n_tiles)):
            nj = tiles_nj[t]
            fap = feat[row0s[t] : row0s[t] + P * nj, :].rearrange(
                "(p j) d -> p j d", p=P, j=nj
            )
            faugs[t] = faug_tile(t)
            nc.gpsimd.dma_start(faugs[t][:, :, :], fap)

        # iota over classes (i32: the one-hot compare stays in DVE 1x mode —
        # 2-port DVE perf modes starve SWDGE descriptor fetch); emitted after
        # the prefetch issues so it doesn't delay the stream start
        iota_i = pconst.tile([P, nj_max, C], I32)
        nc.gpsimd.iota(
            iota_i[:, :, :],
            pattern=[[0, nj_max], [1, C]],
            base=0,
            channel_multiplier=0,
        )

        # merged output: [:, 0:64] sums^T (copied from psum at the end),
        # [:, 64 + 2t] ACT ssq col, [:, 64 + 2t + 1] DVE ssq col per tile
        W = C + 2 * n_tiles
        main_sb = pout.tile([P, W], F32)
        nc.gpsimd.memset(main_sb[:, :], 0.0)

        psum_sumsT = ppsum.tile([P, C], F32)

        mm_idx = 0
        for t in range(n_tiles):
            nj = tiles_nj[t]
            s0 = s0s[t]
            faug = faugs.pop(t)

            sfx = "" if nj == NJ else f"_tail{nj}"
            tb = None if nj == NJ else nj_count[nj]
            oh = poh.tile([P, nj, C], BF16, tag="oh" + sfx, bufs=tb)
            nc.vector.tensor_tensor(
                oh[:, :, :],
                iota_i[:, 0:nj, :],
                tgt_i[:, s0 : s0 + nj].broadcast_to([P, nj, C]),
                op=mybir.AluOpType.is_equal,
            )

            # sum of squares: ACT leading subtiles, DVE the rest. For the
            # last few tiles split half/half so the post-stream squares run
            # on both engines in parallel (shortest drain).
            if t >= n_tiles - 5:
                s_sp = nj // 2
                sfx = f"_dr{t}"
                tb = 1
            else:
                s_sp = min(nj, SQ_ACT)
            if s_sp:
                sqa = psq.tile([P, s_sp, D], BF16, tag="sqa" + sfx, bufs=tb)
                nc.scalar.activation(
                    sqa[:, :, :],
                    faug[:, 0:s_sp, :],
                    mybir.ActivationFunctionType.Square,
                    accum_out=main_sb[:, C + 2 * t : C + 2 * t + 1],
                )
            if nj > s_sp:
                sqb = psq.tile([P, nj - s_sp, D], BF16, tag="sqb" + sfx,
                               bufs=tb)
                nc.vector.scalar_tensor_tensor(
                    out=sqb[:, :, :],
                    in0=faug[:, s_sp:nj, :],
                    scalar=1.0,
                    in1=faug[:, s_sp:nj, :],
                    op0=mybir.AluOpType.mult,
                    op1=mybir.AluOpType.mult,
                    accum_out=main_sb[:, C + 2 * t + 1 : C + 2 * t + 2],
                )

            for j in range(nj):
                nc.tensor.matmul(
                    psum_sumsT[:, :],
                    lhsT=faug[:, j, :],
                    rhs=oh[:, j, :],
                    start=(mm_idx == 0),
                    stop=(mm_idx == n_mm - 1),
                )
                mm_idx += 1

            tn = t + PREFETCH
            if tn < n_tiles:
                njn = tiles_nj[tn]
                fap = feat[row0s[tn] : row0s[tn] + P * njn, :].rearrange(
                    "(p j) d -> p j d", p=P, j=njn
                )
                faugs[tn] = faug_tile(tn)
                nc.gpsimd.dma_start(faugs[tn][:, :, :], fap)

        nc.vector.tensor_copy(main_sb[:, 0:C], psum_sumsT[:, :])
        nc.sync.dma_start(out_main[:, :], main_sb[:, :])


def build_program(tiles_nj):
    # Bacc (not raw Bass): its compile() runs generate_event_semaphores,
    # which splits multi-semaphore waits into EventSemaphore instructions —
    # TRN2 instructions (notably direct-2D DMAs) carry at most one wait.
    nc = bacc.Bacc()
    n_tiles = len(tiles_nj)
    rows = P * sum(tiles_nj)
    feat = nc.dram_tensor("features", [rows, D], F32, kind="ExternalInput")
    tgt = nc.dram_tensor("targets", [P, S_TOTAL], I32, kind="ExternalInput")
    out_main = nc.dram_tensor("out_main", [P, C + 2 * n_tiles], F32,
                              kind="ExternalOutput")
    with tile.TileContext(nc) as tc:
        kernel_body(
            tc,
            (out_main[:, :],),
            (feat[:, :], tgt[:, :]),
            tiles_nj,
        )
    nc.compile()
    return nc


_PROGRAM = None


def _get_program():
    global _PROGRAM
    if _PROGRAM is None:
        _PROGRAM = build_program(TILES_NJ)
    return _PROGRAM


def make_in_maps(features, targets):
    features = np.asarray(features)
    targets = np.asarray(targets)
    if features.dtype != np.float32:
        features = features.astype(np.float32)
    if targets.dtype != np.int32:
        targets = targets.astype(np.int32)
    in_maps = []
    b = features.shape[0]
    for i in range(N_CORES):
        lo = i * ROWS_PER_CORE
        hi = min((i + 1) * ROWS_PER_CORE, b)
        f = features[lo:hi]
        t = targets[lo:hi]
        pad = ROWS_PER_CORE - (hi - lo)
        if pad:
            f = np.concatenate([f, np.zeros((pad, D), np.float32)])
            t = np.concatenate([t, np.full((pad,), PAD_CLASS, np.int32)])
        # block the targets to the per-tile (p, j) layout: tile t's subtile
        # column s0+j on partition p holds target[row0 + p*nj + j]
        tb = np.empty((P, S_TOTAL), np.int32)
        r = s = 0
        for nj in TILES_NJ:
            tb[:, s : s + nj] = t[r : r + P * nj].reshape(P, nj)
            r += P * nj
            s += nj
        in_maps.append({"features": f, "targets": tb})
    return in_maps


def reduce_partials(main_parts, counts, b):
    """main_parts: [P, C + 2*n_tiles] per core (sums^T cols 0:64, then ssq
    accumulator columns); counts: exact per-class counts from the host."""
    sums = np.zeros((C, D), np.float64)
    ssq = 0.0
    for m in main_parts:
        m = m.astype(np.float64)
        sums += m[:, 0:C].T
        ssq += float(m[:, C:].sum())

    counts_c = np.maximum(counts, 1.0)
    centers = sums / counts_c[:, None]
    intra = (
        ssq
        - 2.0 * float((sums * centers).sum())
        + float((counts * (centers**2).sum(axis=1)).sum())
    ) / b

    gram = centers @ centers.T
    n2 = np.diag(gram)
    d2 = n2[:, None] + n2[None, :] - 2.0 * gram
    hinge = np.maximum(MARGIN - d2, 0.0)
    w = np.ones((C, C))
    w[1, 2] = 2.0
    upper = np.triu(np.ones((C, C)), k=1)
    n_pairs = C * (C - 1) // 2
    inter = float((w * hinge * upper).sum()) / n_pairs
    return np.float32(intra + inter)


def exec_prestaged(nc, in_maps):
    """Execute the prebuilt Bass module on 8 cores with explicit input
    staging: device_put every shard and block until it lands in HBM BEFORE
    launching. The stock jit path lets a device start executing while other
    devices' host->HBM input transfers are still in flight on shared HBM
    stacks, which randomly robs 15-20% of stream bandwidth from 1-3 cores
    per run. Same lowering as concourse.bass2jax.run_bass_via_pjrt.
    """
    import jax
    from jax.experimental.shard_map import shard_map
    from jax.sharding import Mesh, NamedSharding, PartitionSpec

    from concourse import bass2jax as b2j

    n_cores = len(in_maps)
    b2j.install_neuronx_cc_hook()

    partition_name = (
        nc.partition_id_tensor.name if nc.partition_id_tensor else None
    )
    in_names, out_names, out_avals, zero_outs = [], [], [], []
    for alloc in nc.m.functions[0].allocations:
        if not isinstance(alloc, mybir.MemoryLocationSet):
            continue
        name = alloc.memorylocations[0].name
        if alloc.kind == "ExternalInput":
            if name != partition_name:
                in_names.append(name)
        elif alloc.kind == "ExternalOutput":
            out_names.append(name)
            shape = tuple(alloc.tensor_shape)
            dtype = mybir.dt.np(alloc.dtype)
            out_avals.append(jax.core.ShapedArray(shape, dtype))
            zero_outs.append(np.zeros(shape, dtype))
    n_params = len(in_names)
    n_outs = len(out_avals)
    in_names.extend(out_names)
    if partition_name is not None:
        in_names.append(partition_name)
    donate = tuple(range(n_params, n_params + n_outs))

    def _body(*args):
        operands = list(args)
        if partition_name is not None:
            operands.append(b2j.partition_id_tensor())
        outs = b2j._bass_exec_p.bind(
            *operands,
            out_avals=tuple(out_avals),
            in_names=tuple(in_names),
            out_names=tuple(out_names),
            lowering_input_output_aliases=(),
            sim_require_finite=True,
            sim_require_nnan=True,
            nc=nc,
        )
        return tuple(outs)

    devices = jax.devices()[:n_cores]
    mesh = Mesh(np.asarray(devices), ("core",))
    in_specs = (PartitionSpec("core"),) * (n_params + n_outs)
    out_specs = (PartitionSpec("core"),) * len(out_names)
    sharded = jax.jit(
        shard_map(_body, mesh=mesh, in_specs=in_specs, out_specs=out_specs,
                  check_rep=False),
        donate_argnums=donate,
        keep_unused=True,
    )
    per_core = [
        [np.asarray(m[name]) for name in in_names[:n_params]] for m in in_maps
    ]
    concat_in = [
        np.concatenate([per_core[c][i] for c in range(n_cores)], axis=0)
        for i in range(n_params)
    ]
    concat_zeros = [
        np.zeros((n_cores * z.shape[0], *z.shape[1:]), z.dtype)
        for z in zero_outs
    ]
    sharding = NamedSharding(mesh, PartitionSpec("core"))
    staged = jax.device_put(concat_in + concat_zeros, [sharding] * (n_params + n_outs))
    jax.block_until_ready(staged)
    out_arrs = sharded(*staged)
    jax.block_until_ready(out_arrs)
    return [
        {
            name: np.asarray(out_arrs[i]).reshape(n_cores, *out_avals[i].shape)[c]
            for i, name in enumerate(out_names)
        }
        for c in range(n_cores)
    ]


def run(features, targets, trace=False, trace_cores=None, prestage=True, **kw):
    nc = _get_program()
    in_maps = make_in_maps(features, targets)
    if prestage and not trace:
        try:
            results = exec_prestaged(nc, in_maps)
        except Exception:
            res = run_bass_kernel_spmd(
                nc, in_maps, core_ids=list(range(N_CORES))
            )
            results = res.results
            targets = np.asarray(targets)
            counts = np.bincount(
                targets.astype(np.int64).reshape(-1), minlength=C
            )[:C].astype(np.float64)
            out = reduce_partials(
                [r["out_main"] for r in results],
                counts,
                np.asarray(features).shape[0],
            )
            return out, res

        class _Res:
            pass

        res = _Res()
        res.results = results
        res.exec_time_ns = None
        res.mean_exec_time_ns = None
        res.max_exec_time_core_id = None
        res.instructions_and_trace = None
    else:
        res = run_bass_kernel_spmd(
            nc,
            in_maps,
            core_ids=list(range(N_CORES)),
            trace=trace,
            trace_cores=trace_cores,
            **kw,
        )
    targets = np.asarray(targets)
    counts = np.bincount(
        targets.astype(np.int64).reshape(-1), minlength=C
    )[:C].astype(np.float64)
    out = reduce_partials(
        [r["out_main"] for r in res.results],
        counts,
        np.asarray(features).shape[0],
    )
    return out, res


def kernel(features, targets):
    out, _ = run(features, targets)
    return np.array(out, dtype=np.float32)
